# revision 6
# baseline (speedup 1.0000x reference)
"""Bidirectional GRU classifier kernel for Trainium2 (8 NeuronCores).

Strategy (v3):
  - Direction parallel + time-sharded: cores 0-3 forward GRU, cores 4-7
    backward GRU (forward scan over time-reversed input); single SPMD
    program, per-core differences live in the input data.
  - Each core owns 1024 output steps split into M_CHUNKS=32 chunks processed
    as parallel columns (42 serial steps incl. 10 warmup); 2 anti-phased
    recurrence chains of 512 columns each.
  - bf16 datapath: x, h, r, z, n and all matmul operands bf16 (PE 1 cyc/row,
    DVE 2x_1p on SBUF elementwise ops, half DMA); PSUM accum stays fp32.
  - Per chain-step:
      pr = W_r x_t + U_r v_{t-1} - U_r u_{t-1}   (v-split: no wait on h')
      pz = W_z x_t + U_z h_{t-1}
      r = sigmoid(pr + b_r), z = sigmoid(pz + b_z)       [ACT]
      B = (phn + b_hn) * r                               [DVE STT -> PSUM]
      B += W_n x_t                                       [PE accumulate]
      n = tanh(B + b_in)                                 [ACT]
      zm1 = z - 1                                        [DVE tensor_scalar]
      u = zm1 * n                                        [DVE TT bf16 2x]
      v = z * h_prev                                     [Pool]
      h' = v - u = z*h + (1-z)*n                         [DVE TT]
  - No FC on device: h streamed out bf16; FC (K=10) + direction sum + b_fc
    run on host.
"""

import sys

sys.path.insert(0, "/opt/trn_rl_repo")

import numpy as np
import ml_dtypes

# Problem constants
B, T, DX, H, K = 32, 4096, 128, 128, 10
N_CORES = 8
CORES_PER_DIR = 4

# Sharding parameters
M_CHUNKS = 32       # chunks per core
N_CHAINS = 2        # independent recurrence chains per core
C_STEPS = 1024 // M_CHUNKS  # output steps per chunk
L_WARM = 8          # warmup steps per chunk
STEPS = C_STEPS + L_WARM    # compute steps per chunk
COLS = 32 * M_CHUNKS        # total columns per step (batch x chunks)
XBLK = 4            # x-stream block: steps per DMA block
STAGGER = 5         # serial copies to anti-phase chain 1


def build_gru_program(tc, ins, outs, steps, m_chunks, n_chains, xblk=XBLK):
    """Emit the Tile program. ins/outs: dict name -> bass.AP (DRAM)."""
    import concourse.mybir as mybir
    from contextlib import ExitStack

    nc = tc.nc
    f32 = mybir.dt.float32
    bf16 = mybir.dt.bfloat16
    cols = 32 * m_chunks            # per step, all chains
    cc = cols // n_chains           # per chain
    AF = mybir.ActivationFunctionType
    OP = mybir.AluOpType

    ctx = ExitStack()
    consts = ctx.enter_context(tc.tile_pool(name="consts", bufs=1))
    xpool = ctx.enter_context(tc.tile_pool(name="xblk", bufs=3))
    hpool = ctx.enter_context(tc.tile_pool(name="hbuf", bufs=3))
    spool = ctx.enter_context(tc.tile_pool(name="work", bufs=2))
    pRZ = ctx.enter_context(tc.tile_pool(name="pRZ", bufs=1, space="PSUM"))
    pN = ctx.enter_context(tc.tile_pool(name="pN", bufs=1, space="PSUM"))

    # Load weights/constants once (all bf16 except biases)
    wih = consts.tile([128, 3 * H], bf16, tag="wih")
    nc.sync.dma_start(wih[:], ins["wih_t"][:])
    whh = consts.tile([128, 3 * H], bf16, tag="whh")
    nc.sync.dma_start(whh[:], ins["whh_t"][:])
    u_r_neg = consts.tile([128, H], bf16, tag="urneg")
    nc.sync.dma_start(u_r_neg[:], ins["u_r_neg"][:])
    bias = consts.tile([128, 4], f32, tag="bias")
    nc.sync.dma_start(bias[:], ins["bias"][:])
    b_r, b_z, b_in, b_hn = (bias[:, i : i + 1] for i in range(4))

    w_r, w_z, w_n = (wih[:, g * H : (g + 1) * H] for g in range(3))
    u_r, u_z, u_n = (whh[:, g * H : (g + 1) * H] for g in range(3))

    h_init = consts.tile([128, cols], bf16, tag="hinit")
    nc.sync.dma_start(h_init[:], ins["zeros"][:])

    x_dram = ins["x_t"]
    # h viewed as [128, steps, cols] for strided per-chain pair stores
    h_dram = outs["h_out"].rearrange("p (t c) -> p t c", c=cols)

    # persistent per-chain PSUM banks (4 per chain = 8 total)
    prb = [pRZ.tile([128, cc], f32, tag=f"prb{c}", name=f"prb{c}")
           for c in range(n_chains)]
    pzb = [pRZ.tile([128, cc], f32, tag=f"pzb{c}", name=f"pzb{c}")
           for c in range(n_chains)]
    phn = [pN.tile([128, cc], f32, tag=f"phn{c}", name=f"phn{c}")
           for c in range(n_chains)]
    pB = [pN.tile([128, cc], f32, tag=f"pB{c}", name=f"pB{c}")
          for c in range(n_chains)]

    xtiles = {}
    h_prev = [h_init[:, c * cc : (c + 1) * cc] for c in range(n_chains)]
    # stagger chain 1 by ~half a step period (serial copy chain)
    if n_chains == 2:
        stag = h_prev[1]
        for s in range(STAGGER):
            nxt = consts.tile([128, cc], bf16, tag=f"stag{s}", name=f"stag{s}")
            nc.vector.tensor_copy(nxt[:], stag)
            stag = nxt[:]
        h_prev[1] = stag
    h_pair = [None] * n_chains
    # deferred v-split accumulation operands for the NEXT step's pr
    vu_accum = [None] * n_chains

    def get_block(bp):
        if bp not in xtiles:
            bsteps = min(xblk, steps - bp * xblk)
            xt_blk = xpool.tile([128, bsteps * cols], bf16, tag="xblk",
                                name=f"xblk_{bp}")
            nc.sync.dma_start(
                xt_blk[:], x_dram[:, bp * xblk * cols : (bp * xblk + bsteps) * cols]
            )
            xtiles[bp] = xt_blk
            for stale in [k for k in xtiles if k < bp - 2]:
                del xtiles[stale]
        return xtiles[bp]

    def x_step(tp, c):
        """[128, cc] moving operand of x for step tp, chain c."""
        xt_b = get_block(tp // xblk)
        v = xt_b[:].rearrange("p (s c) -> p s c", c=cols)
        return v[:, tp % xblk, c * cc : (c + 1) * cc]

    for t in range(steps):
        get_block(t // xblk)

        for c in range(n_chains):
            hp = h_prev[c]

            if t % 2 == 0:
                h_pair[c] = hpool.tile([128, 2 * cc], bf16,
                                       tag=f"hpair{c}", name=f"hpair{c}_{t}")

            # x-side projections for this step (reset banks; they only wait
            # on last step's sigmoid reads, well off the critical path)
            xs = x_step(t, c)
            nc.tensor.matmul(prb[c][:], w_r, xs, start=True, stop=False,
                             skip_group_check=True)
            nc.tensor.matmul(pzb[c][:], w_z, xs, start=True, stop=False,
                             skip_group_check=True)

            # r-gate hidden contribution: deferred v/u split from the
            # previous step (or plain U_r @ h on the first step).
            if vu_accum[c] is not None:
                v_prev, u_prev = vu_accum[c]
                nc.tensor.matmul(prb[c][:], u_r, v_prev, start=False,
                                 stop=False, skip_group_check=True)
                nc.tensor.matmul(prb[c][:], u_r_neg[:], u_prev, start=False,
                                 stop=True, skip_group_check=True)
            else:
                nc.tensor.matmul(prb[c][:], u_r, hp, start=False, stop=True,
                                 skip_group_check=True)
            nc.tensor.matmul(phn[c][:], u_n, hp, start=True, stop=True)
            nc.tensor.matmul(pzb[c][:], u_z, hp, start=False, stop=True,
                             skip_group_check=True)

            r_t = spool.tile([128, cc], bf16, tag=f"r{c}")
            nc.scalar.activation(r_t[:], prb[c][:], AF.Sigmoid, bias=b_r)
            z_t = spool.tile([128, cc], bf16, tag=f"z{c}")
            nc.scalar.activation(z_t[:], pzb[c][:], AF.Sigmoid, bias=b_z)

            # B = (phn + b_hn) * r  (single fused DVE op into PSUM)
            nc.vector.scalar_tensor_tensor(pB[c][:], phn[c][:], b_hn, r_t[:],
                                           OP.add, OP.mult)
            # B += W_n @ x_t  (PE accumulate onto DVE-written bank)
            nc.tensor.matmul(pB[c][:], w_n, x_step(t, c), start=False,
                             stop=True, skip_group_check=True)
            # n = tanh(B + b_in)
            n_t = spool.tile([128, cc], bf16, tag=f"n{c}")
            nc.scalar.activation(n_t[:], pB[c][:], AF.Tanh, bias=b_in)

            # v = z * h_prev  (Pool engine, off critical path)
            v_t = spool.tile([128, cc], bf16, tag=f"v{c}")
            nc.gpsimd.tensor_mul(v_t[:], z_t[:], hp)

            # u = (z - 1) * n ; h' = v - u = z*h + (1-z)*n
            u_t = spool.tile([128, cc], bf16, tag=f"u{c}")
            nc.vector.scalar_tensor_tensor(u_t[:], z_t[:], 1.0, n_t[:],
                                           OP.subtract, OP.mult)
            half = (t % 2) * cc
            h_new = h_pair[c][:, half : half + cc]
            nc.vector.tensor_sub(h_new, v_t[:], u_t[:])
            h_prev[c] = h_new
            vu_accum[c] = (v_t[:], u_t[:])

            if t % 2 == 1:
                hv = h_pair[c][:].rearrange("p (t c) -> p t c", c=cc)
                nc.sync.dma_start(
                    h_dram[:, t - 1 : t + 1, c * cc : (c + 1) * cc], hv
                )

    ctx.close()


def _declare_io(nc, steps, m_chunks):
    import concourse.mybir as mybir

    cols = 32 * m_chunks
    f32 = mybir.dt.float32
    bf16 = mybir.dt.bfloat16
    ins = {
        "x_t": nc.dram_tensor("x_t", [128, steps * cols], bf16, kind="ExternalInput").ap(),
        "wih_t": nc.dram_tensor("wih_t", [128, 3 * H], bf16, kind="ExternalInput").ap(),
        "whh_t": nc.dram_tensor("whh_t", [128, 3 * H], bf16, kind="ExternalInput").ap(),
        "u_r_neg": nc.dram_tensor("u_r_neg", [128, H], bf16, kind="ExternalInput").ap(),
        "bias": nc.dram_tensor("bias", [128, 4], f32, kind="ExternalInput").ap(),
        "zeros": nc.dram_tensor("zeros", [128, cols], bf16, kind="ExternalInput").ap(),
    }
    outs = {
        "h_out": nc.dram_tensor(
            "h_out", [128, steps * cols], bf16, kind="ExternalOutput"
        ).ap(),
    }
    return ins, outs


def build_module(steps=STEPS, m_chunks=M_CHUNKS, n_chains=N_CHAINS):
    import concourse.bacc as bacc
    import concourse.tile as tile

    nc = bacc.Bacc("TRN2", target_bir_lowering=False, debug=False)
    ins, outs = _declare_io(nc, steps, m_chunks)
    with tile.TileContext(nc) as tc:
        build_gru_program(tc, ins, outs, steps, m_chunks, n_chains)
    nc.compile()
    return nc


# ---------------- host-side data prep / assembly ----------------

def chunk_starts(n_segments, c_steps, l_warm):
    """Compute-range start per global segment (clamped at 0)."""
    return [max(0, s * c_steps - l_warm) for s in range(n_segments)]


def prep_core_inputs(x_dir, wih, whh, bih, bhh, core, steps, m_chunks,
                     c_steps, l_warm):
    """Build the input map for one core of one direction.

    x_dir: [B, T, DX] (already time-reversed for the backward direction)
    wih/whh: [3H, {DX,H}], bih/bhh: [3H]
    """
    cols = 32 * m_chunks
    starts = chunk_starts(CORES_PER_DIR * m_chunks, c_steps, l_warm)
    xt = np.empty((128, steps, m_chunks, B), np.float32)
    for j in range(m_chunks):
        g = starts[core * m_chunks + j]
        xt[:, :, j, :] = np.transpose(x_dir[:, g : g + steps, :], (2, 1, 0))
    bias = np.zeros((128, 4), np.float32)
    bias[:, 0] = bih[0:H] + bhh[0:H]          # r
    bias[:, 1] = bih[H : 2 * H] + bhh[H : 2 * H]  # z
    bias[:, 2] = bih[2 * H : 3 * H]           # input-side n bias (tanh bias)
    bias[:, 3] = bhh[2 * H : 3 * H]           # hidden-side n bias (STT scalar)
    bf = ml_dtypes.bfloat16
    return {
        "x_t": np.ascontiguousarray(xt.reshape(128, steps * cols)).astype(bf),
        "wih_t": np.ascontiguousarray(wih.T).astype(bf),     # [DX, 3H]
        "whh_t": np.ascontiguousarray(whh.T).astype(bf),     # [H, 3H]
        "u_r_neg": np.ascontiguousarray(-whh[0:H, :].T).astype(bf),  # [H, H]
        "bias": bias,
        "zeros": np.zeros((128, cols), bf),
    }


def assemble_direction(h_parts, steps, m_chunks, c_steps, l_warm):
    """h_parts: list over CORES_PER_DIR cores of [128, steps*cols] bf16.
    Returns [B, T, H] hidden states for this direction (pre-reversal)."""
    out = np.empty((B, T, H), np.float32)
    for core in range(CORES_PER_DIR):
        hp = np.asarray(h_parts[core]).reshape(H, steps, m_chunks, B)
        for j in range(m_chunks):
            s = core * m_chunks + j
            off = s * c_steps - max(0, s * c_steps - l_warm)  # warmup offset
            seg = hp[:, off : off + c_steps, j, :].astype(np.float32)
            out[:, s * c_steps : (s + 1) * c_steps, :] = np.transpose(seg, (2, 1, 0))
    return out


_COMPILED = {}


def _get_module(steps, m_chunks):
    key = (steps, m_chunks)
    if key not in _COMPILED:
        _COMPILED[key] = build_module(steps, m_chunks)
    return _COMPILED[key]


def make_in_maps(x, W_ih_f, W_hh_f, b_ih_f, b_hh_f, W_ih_b, W_hh_b, b_ih_b,
                 b_hh_b):
    x = np.asarray(x, np.float32)
    x_rev = x[:, ::-1, :]
    in_maps = []
    for core in range(CORES_PER_DIR):
        in_maps.append(prep_core_inputs(
            x, W_ih_f, W_hh_f, b_ih_f, b_hh_f, core,
            STEPS, M_CHUNKS, C_STEPS, L_WARM))
    for core in range(CORES_PER_DIR):
        in_maps.append(prep_core_inputs(
            x_rev, W_ih_b, W_hh_b, b_ih_b, b_hh_b, core,
            STEPS, M_CHUNKS, C_STEPS, L_WARM))
    return in_maps


LAST_RES = None


def kernel(x, W_ih_f, W_hh_f, b_ih_f, b_hh_f, W_ih_b, W_hh_b, b_ih_b, b_hh_b,
           W_fc, b_fc):
    global LAST_RES
    from concourse.bass_utils import run_bass_kernel_spmd

    nc = _get_module(STEPS, M_CHUNKS)
    in_maps = make_in_maps(x, W_ih_f, W_hh_f, b_ih_f, b_hh_f,
                           W_ih_b, W_hh_b, b_ih_b, b_hh_b)
    res = run_bass_kernel_spmd(nc, in_maps, core_ids=list(range(N_CORES)))
    LAST_RES = res

    hf = assemble_direction([res.results[c]["h_out"] for c in range(4)],
                            STEPS, M_CHUNKS, C_STEPS, L_WARM)
    hb_rev = assemble_direction([res.results[c]["h_out"] for c in range(4, 8)],
                                STEPS, M_CHUNKS, C_STEPS, L_WARM)
    hb = hb_rev[:, ::-1, :]
    W_fc = np.asarray(W_fc, np.float32)
    y = hf @ W_fc[:, 0:H].T + hb @ W_fc[:, H : 2 * H].T
    return (y + np.asarray(b_fc, np.float32)).astype(np.float32)


# revision 7
# speedup vs baseline: 1.0971x; 1.0971x over previous
"""Bidirectional GRU classifier kernel for Trainium2 (8 NeuronCores).

Strategy (v3):
  - Direction parallel + time-sharded: cores 0-3 forward GRU, cores 4-7
    backward GRU (forward scan over time-reversed input); single SPMD
    program, per-core differences live in the input data.
  - Each core owns 1024 output steps split into M_CHUNKS=32 chunks processed
    as parallel columns (42 serial steps incl. 10 warmup); 2 anti-phased
    recurrence chains of 512 columns each.
  - bf16 datapath: x, h, r, z, n and all matmul operands bf16 (PE 1 cyc/row,
    DVE 2x_1p on SBUF elementwise ops, half DMA); PSUM accum stays fp32.
  - Per chain-step:
      pr = W_r x_t + U_r v_{t-1} - U_r u_{t-1}   (v-split: no wait on h')
      pz = W_z x_t + U_z h_{t-1}
      r = sigmoid(pr + b_r), z = sigmoid(pz + b_z)       [ACT]
      B = (phn + b_hn) * r                               [DVE STT -> PSUM]
      B += W_n x_t                                       [PE accumulate]
      n = tanh(B + b_in)                                 [ACT]
      zm1 = z - 1                                        [DVE tensor_scalar]
      u = zm1 * n                                        [DVE TT bf16 2x]
      v = z * h_prev                                     [Pool]
      h' = v - u = z*h + (1-z)*n                         [DVE TT]
  - No FC on device: h streamed out bf16; FC (K=10) + direction sum + b_fc
    run on host.
"""

import sys

sys.path.insert(0, "/opt/trn_rl_repo")

import numpy as np
import ml_dtypes

# Problem constants
B, T, DX, H, K = 32, 4096, 128, 128, 10
N_CORES = 8
CORES_PER_DIR = 4

# Sharding parameters
M_CHUNKS = 32       # chunks per core
N_CHAINS = 2        # independent recurrence chains per core
C_STEPS = 1024 // M_CHUNKS  # output steps per chunk
L_WARM = 8          # warmup steps per chunk
STEPS = C_STEPS + L_WARM    # compute steps per chunk
COLS = 32 * M_CHUNKS        # total columns per step (batch x chunks)
XBLK = 4            # x-stream block: steps per DMA block
STAGGER = 5         # serial copies to anti-phase chain 1


def build_gru_program(tc, ins, outs, steps, m_chunks, n_chains, xblk=XBLK):
    """Emit the Tile program. ins/outs: dict name -> bass.AP (DRAM)."""
    import concourse.mybir as mybir
    from contextlib import ExitStack

    nc = tc.nc
    f32 = mybir.dt.float32
    bf16 = mybir.dt.bfloat16
    cols = 32 * m_chunks            # per step, all chains
    cc = cols // n_chains           # per chain
    AF = mybir.ActivationFunctionType
    OP = mybir.AluOpType

    ctx = ExitStack()
    consts = ctx.enter_context(tc.tile_pool(name="consts", bufs=1))
    xpool = ctx.enter_context(tc.tile_pool(name="xblk", bufs=3))
    hpool = ctx.enter_context(tc.tile_pool(name="hbuf", bufs=3))
    spool = ctx.enter_context(tc.tile_pool(name="work", bufs=2))
    pRZ = ctx.enter_context(tc.tile_pool(name="pRZ", bufs=1, space="PSUM"))
    pN = ctx.enter_context(tc.tile_pool(name="pN", bufs=1, space="PSUM"))

    # Load weights/constants once (all bf16 except biases)
    wih = consts.tile([128, 3 * H], bf16, tag="wih")
    nc.sync.dma_start(wih[:], ins["wih_t"][:])
    whh = consts.tile([128, 3 * H], bf16, tag="whh")
    nc.sync.dma_start(whh[:], ins["whh_t"][:])
    u_r_neg = consts.tile([128, H], bf16, tag="urneg")
    nc.sync.dma_start(u_r_neg[:], ins["u_r_neg"][:])
    bias = consts.tile([128, 4], f32, tag="bias")
    nc.sync.dma_start(bias[:], ins["bias"][:])
    b_r, b_z, b_in, b_hn = (bias[:, i : i + 1] for i in range(4))

    w_r, w_z, w_n = (wih[:, g * H : (g + 1) * H] for g in range(3))
    u_r, u_z, u_n = (whh[:, g * H : (g + 1) * H] for g in range(3))

    h_init = consts.tile([128, cols], bf16, tag="hinit")
    nc.sync.dma_start(h_init[:], ins["zeros"][:])

    x_dram = ins["x_t"]
    # h viewed as [128, steps, cols] for strided per-chain pair stores
    h_dram = outs["h_out"].rearrange("p (t c) -> p t c", c=cols)

    # persistent per-chain PSUM banks (4 per chain = 8 total)
    prb = [pRZ.tile([128, cc], f32, tag=f"prb{c}", name=f"prb{c}")
           for c in range(n_chains)]
    pzb = [pRZ.tile([128, cc], f32, tag=f"pzb{c}", name=f"pzb{c}")
           for c in range(n_chains)]
    phn = [pN.tile([128, cc], f32, tag=f"phn{c}", name=f"phn{c}")
           for c in range(n_chains)]
    pB = [pN.tile([128, cc], f32, tag=f"pB{c}", name=f"pB{c}")
          for c in range(n_chains)]

    xtiles = {}
    h_prev = [h_init[:, c * cc : (c + 1) * cc] for c in range(n_chains)]
    # stagger chain 1 by ~half a step period (serial copy chain)
    if n_chains == 2:
        stag = h_prev[1]
        for s in range(STAGGER):
            nxt = consts.tile([128, cc], bf16, tag=f"stag{s}", name=f"stag{s}")
            nc.vector.tensor_copy(nxt[:], stag)
            stag = nxt[:]
        h_prev[1] = stag
    h_pair = [None] * n_chains
    # deferred v-split accumulation operands for the NEXT step's pr
    vu_accum = [None] * n_chains

    def get_block(bp):
        if bp not in xtiles:
            bsteps = min(xblk, steps - bp * xblk)
            xt_blk = xpool.tile([128, bsteps * cols], bf16, tag="xblk",
                                name=f"xblk_{bp}")
            nc.sync.dma_start(
                xt_blk[:], x_dram[:, bp * xblk * cols : (bp * xblk + bsteps) * cols]
            )
            xtiles[bp] = xt_blk
            for stale in [k for k in xtiles if k < bp - 2]:
                del xtiles[stale]
        return xtiles[bp]

    def x_step(tp, c):
        """[128, cc] moving operand of x for step tp, chain c."""
        xt_b = get_block(tp // xblk)
        v = xt_b[:].rearrange("p (s c) -> p s c", c=cols)
        return v[:, tp % xblk, c * cc : (c + 1) * cc]

    for t in range(steps):
        get_block(t // xblk)

        for c in range(n_chains):
            hp = h_prev[c]

            if t % 2 == 0:
                h_pair[c] = hpool.tile([128, 2 * cc], bf16,
                                       tag=f"hpair{c}", name=f"hpair{c}_{t}")

            # x-side projections for this step (reset banks; they only wait
            # on last step's sigmoid reads, well off the critical path)
            xs = x_step(t, c)
            nc.tensor.matmul(prb[c][:], w_r, xs, start=True, stop=False,
                             skip_group_check=True)
            nc.tensor.matmul(pzb[c][:], w_z, xs, start=True, stop=False,
                             skip_group_check=True)

            # r-gate hidden contribution: deferred v/u split from the
            # previous step (or plain U_r @ h on the first step).
            if vu_accum[c] is not None:
                v_prev, u_prev = vu_accum[c]
                nc.tensor.matmul(prb[c][:], u_r, v_prev, start=False,
                                 stop=False, skip_group_check=True)
                nc.tensor.matmul(prb[c][:], u_r_neg[:], u_prev, start=False,
                                 stop=True, skip_group_check=True)
            else:
                nc.tensor.matmul(prb[c][:], u_r, hp, start=False, stop=True,
                                 skip_group_check=True)
            nc.tensor.matmul(phn[c][:], u_n, hp, start=True, stop=True)
            nc.tensor.matmul(pzb[c][:], u_z, hp, start=False, stop=True,
                             skip_group_check=True)

            r_t = spool.tile([128, cc], bf16, tag=f"r{c}")
            nc.scalar.activation(r_t[:], prb[c][:], AF.Sigmoid, bias=b_r)
            z_t = spool.tile([128, cc], bf16, tag=f"z{c}")
            nc.scalar.activation(z_t[:], pzb[c][:], AF.Sigmoid, bias=b_z)

            # B = (phn + b_hn) * r  (single fused DVE op into PSUM)
            nc.vector.scalar_tensor_tensor(pB[c][:], phn[c][:], b_hn, r_t[:],
                                           OP.add, OP.mult)
            # B += W_n @ x_t  (PE accumulate onto DVE-written bank)
            nc.tensor.matmul(pB[c][:], w_n, x_step(t, c), start=False,
                             stop=True, skip_group_check=True)
            # n = tanh(B + b_in)
            n_t = spool.tile([128, cc], bf16, tag=f"n{c}")
            nc.scalar.activation(n_t[:], pB[c][:], AF.Tanh, bias=b_in)

            # zm1 = z - 1 (off critical path, fast tensor_scalar)
            zm1 = spool.tile([128, cc], bf16, tag=f"zm1{c}")
            nc.vector.tensor_scalar_add(zm1[:], z_t[:], -1.0)
            # v = z * h_prev  (Pool engine, off critical path)
            v_t = spool.tile([128, cc], bf16, tag=f"v{c}")
            nc.gpsimd.tensor_mul(v_t[:], z_t[:], hp)

            # u = (z-1) * n ; h' = v - u = z*h + (1-z)*n
            u_t = spool.tile([128, cc], bf16, tag=f"u{c}")
            nc.vector.tensor_mul(u_t[:], zm1[:], n_t[:])
            half = (t % 2) * cc
            h_new = h_pair[c][:, half : half + cc]
            nc.vector.tensor_sub(h_new, v_t[:], u_t[:])
            h_prev[c] = h_new
            vu_accum[c] = (v_t[:], u_t[:])

            if t % 2 == 1:
                hv = h_pair[c][:].rearrange("p (t c) -> p t c", c=cc)
                nc.sync.dma_start(
                    h_dram[:, t - 1 : t + 1, c * cc : (c + 1) * cc], hv
                )

    ctx.close()


def _declare_io(nc, steps, m_chunks):
    import concourse.mybir as mybir

    cols = 32 * m_chunks
    f32 = mybir.dt.float32
    bf16 = mybir.dt.bfloat16
    ins = {
        "x_t": nc.dram_tensor("x_t", [128, steps * cols], bf16, kind="ExternalInput").ap(),
        "wih_t": nc.dram_tensor("wih_t", [128, 3 * H], bf16, kind="ExternalInput").ap(),
        "whh_t": nc.dram_tensor("whh_t", [128, 3 * H], bf16, kind="ExternalInput").ap(),
        "u_r_neg": nc.dram_tensor("u_r_neg", [128, H], bf16, kind="ExternalInput").ap(),
        "bias": nc.dram_tensor("bias", [128, 4], f32, kind="ExternalInput").ap(),
        "zeros": nc.dram_tensor("zeros", [128, cols], bf16, kind="ExternalInput").ap(),
    }
    outs = {
        "h_out": nc.dram_tensor(
            "h_out", [128, steps * cols], bf16, kind="ExternalOutput"
        ).ap(),
    }
    return ins, outs


def build_module(steps=STEPS, m_chunks=M_CHUNKS, n_chains=N_CHAINS):
    import concourse.bacc as bacc
    import concourse.tile as tile

    nc = bacc.Bacc("TRN2", target_bir_lowering=False, debug=False)
    ins, outs = _declare_io(nc, steps, m_chunks)
    with tile.TileContext(nc) as tc:
        build_gru_program(tc, ins, outs, steps, m_chunks, n_chains)
    nc.compile()
    return nc


# ---------------- host-side data prep / assembly ----------------

def chunk_starts(n_segments, c_steps, l_warm):
    """Compute-range start per global segment (clamped at 0)."""
    return [max(0, s * c_steps - l_warm) for s in range(n_segments)]


def prep_core_inputs(x_dir, wih, whh, bih, bhh, core, steps, m_chunks,
                     c_steps, l_warm):
    """Build the input map for one core of one direction.

    x_dir: [B, T, DX] (already time-reversed for the backward direction)
    wih/whh: [3H, {DX,H}], bih/bhh: [3H]
    """
    cols = 32 * m_chunks
    starts = chunk_starts(CORES_PER_DIR * m_chunks, c_steps, l_warm)
    xt = np.empty((128, steps, m_chunks, B), np.float32)
    for j in range(m_chunks):
        g = starts[core * m_chunks + j]
        xt[:, :, j, :] = np.transpose(x_dir[:, g : g + steps, :], (2, 1, 0))
    bias = np.zeros((128, 4), np.float32)
    bias[:, 0] = bih[0:H] + bhh[0:H]          # r
    bias[:, 1] = bih[H : 2 * H] + bhh[H : 2 * H]  # z
    bias[:, 2] = bih[2 * H : 3 * H]           # input-side n bias (tanh bias)
    bias[:, 3] = bhh[2 * H : 3 * H]           # hidden-side n bias (STT scalar)
    bf = ml_dtypes.bfloat16
    return {
        "x_t": np.ascontiguousarray(xt.reshape(128, steps * cols)).astype(bf),
        "wih_t": np.ascontiguousarray(wih.T).astype(bf),     # [DX, 3H]
        "whh_t": np.ascontiguousarray(whh.T).astype(bf),     # [H, 3H]
        "u_r_neg": np.ascontiguousarray(-whh[0:H, :].T).astype(bf),  # [H, H]
        "bias": bias,
        "zeros": np.zeros((128, cols), bf),
    }


def assemble_direction(h_parts, steps, m_chunks, c_steps, l_warm):
    """h_parts: list over CORES_PER_DIR cores of [128, steps*cols] bf16.
    Returns [B, T, H] hidden states for this direction (pre-reversal)."""
    out = np.empty((B, T, H), np.float32)
    for core in range(CORES_PER_DIR):
        hp = np.asarray(h_parts[core]).reshape(H, steps, m_chunks, B)
        for j in range(m_chunks):
            s = core * m_chunks + j
            off = s * c_steps - max(0, s * c_steps - l_warm)  # warmup offset
            seg = hp[:, off : off + c_steps, j, :].astype(np.float32)
            out[:, s * c_steps : (s + 1) * c_steps, :] = np.transpose(seg, (2, 1, 0))
    return out


_COMPILED = {}


def _get_module(steps, m_chunks):
    key = (steps, m_chunks)
    if key not in _COMPILED:
        _COMPILED[key] = build_module(steps, m_chunks)
    return _COMPILED[key]


def make_in_maps(x, W_ih_f, W_hh_f, b_ih_f, b_hh_f, W_ih_b, W_hh_b, b_ih_b,
                 b_hh_b):
    x = np.asarray(x, np.float32)
    x_rev = x[:, ::-1, :]
    in_maps = []
    for core in range(CORES_PER_DIR):
        in_maps.append(prep_core_inputs(
            x, W_ih_f, W_hh_f, b_ih_f, b_hh_f, core,
            STEPS, M_CHUNKS, C_STEPS, L_WARM))
    for core in range(CORES_PER_DIR):
        in_maps.append(prep_core_inputs(
            x_rev, W_ih_b, W_hh_b, b_ih_b, b_hh_b, core,
            STEPS, M_CHUNKS, C_STEPS, L_WARM))
    return in_maps


LAST_RES = None


def kernel(x, W_ih_f, W_hh_f, b_ih_f, b_hh_f, W_ih_b, W_hh_b, b_ih_b, b_hh_b,
           W_fc, b_fc):
    global LAST_RES
    from concourse.bass_utils import run_bass_kernel_spmd

    nc = _get_module(STEPS, M_CHUNKS)
    in_maps = make_in_maps(x, W_ih_f, W_hh_f, b_ih_f, b_hh_f,
                           W_ih_b, W_hh_b, b_ih_b, b_hh_b)
    res = run_bass_kernel_spmd(nc, in_maps, core_ids=list(range(N_CORES)))
    LAST_RES = res

    hf = assemble_direction([res.results[c]["h_out"] for c in range(4)],
                            STEPS, M_CHUNKS, C_STEPS, L_WARM)
    hb_rev = assemble_direction([res.results[c]["h_out"] for c in range(4, 8)],
                                STEPS, M_CHUNKS, C_STEPS, L_WARM)
    hb = hb_rev[:, ::-1, :]
    W_fc = np.asarray(W_fc, np.float32)
    y = hf @ W_fc[:, 0:H].T + hb @ W_fc[:, H : 2 * H].T
    return (y + np.asarray(b_fc, np.float32)).astype(np.float32)


# revision 17
# speedup vs baseline: 1.2783x; 1.1652x over previous
"""Bidirectional GRU classifier kernel for Trainium2 (8 NeuronCores).

Strategy:
  - Direction parallel + time-sharded: cores 0-3 forward GRU, cores 4-7
    backward GRU (forward scan over time-reversed input); single SPMD
    program, per-core differences live in the input data.
  - Each core owns 1024 output steps split into M_CHUNKS=32 chunks processed
    as parallel columns (40 serial steps incl. 8 warmup); 2 anti-phased
    recurrence chains of 512 columns each.
  - bf16 datapath: x, h, r, z, n and all matmul operands bf16 (PE 1 cyc/row,
    DVE 2x_1p on SBUF elementwise ops, half DMA); PSUM accum stays fp32.
  - Per chain-step:
      pr = W_r x_t + U_r v_{t-1} - U_r u_{t-1}   (v-split: no wait on h')
      pz = W_z x_t + U_z h_{t-1}
      r = sigmoid(pr + b_r), z = sigmoid(pz + b_z)       [ACT]
      B = (phn + b_hn) * r                               [DVE STT -> PSUM]
      B += W_n x_t                                       [PE accumulate]
      n = tanh(B + b_in)                                 [ACT]
      zm1 = z - 1                                        [DVE tensor_scalar]
      u = zm1 * n                                        [DVE TT bf16 2x]
      v = z * h_prev                                     [Pool]
      h' = v - u = z*h + (1-z)*n                         [DVE TT]
  - No FC on device: h streamed out bf16; FC (K=10) + direction sum + b_fc
    run on host.
"""

import sys

sys.path.insert(0, "/opt/trn_rl_repo")

import numpy as np
import ml_dtypes

# Problem constants
B, T, DX, H, K = 32, 4096, 128, 128, 10
N_CORES = 8
CORES_PER_DIR = 4

# Sharding parameters
M_CHUNKS = 32       # chunks per core
N_CHAINS = 2        # independent recurrence chains per core
C_STEPS = 1024 // M_CHUNKS  # output steps per chunk
L_WARM = 8          # warmup steps per chunk
STEPS = C_STEPS + L_WARM    # compute steps per chunk
COLS = 32 * M_CHUNKS        # total columns per step (batch x chunks)
XBLK = 4            # x-stream block: steps per DMA block
STAGGER = 7         # serial copies to anti-phase chain 1


def build_gru_program(tc, ins, outs, steps, m_chunks, n_chains, xblk=XBLK):
    """Emit the Tile program. ins/outs: dict name -> bass.AP (DRAM)."""
    import concourse.mybir as mybir
    from contextlib import ExitStack

    nc = tc.nc
    f32 = mybir.dt.float32
    bf16 = mybir.dt.bfloat16
    cols = 32 * m_chunks            # per step, all chains
    cc = cols // n_chains           # per chain
    AF = mybir.ActivationFunctionType
    OP = mybir.AluOpType

    ctx = ExitStack()
    consts = ctx.enter_context(tc.tile_pool(name="consts", bufs=1))
    xpool = ctx.enter_context(tc.tile_pool(name="xblk", bufs=3))
    hpool = ctx.enter_context(tc.tile_pool(name="hbuf", bufs=3))
    spool = ctx.enter_context(tc.tile_pool(name="work", bufs=2))
    pRZ = ctx.enter_context(tc.tile_pool(name="pRZ", bufs=1, space="PSUM"))
    pN = ctx.enter_context(tc.tile_pool(name="pN", bufs=1, space="PSUM"))

    # Load weights/constants once (all bf16 except biases)
    wih = consts.tile([128, 3 * H], bf16, tag="wih")
    nc.sync.dma_start(wih[:], ins["wih_t"][:])
    whh = consts.tile([128, 3 * H], bf16, tag="whh")
    nc.sync.dma_start(whh[:], ins["whh_t"][:])
    u_r_neg = consts.tile([128, H], bf16, tag="urneg")
    nc.sync.dma_start(u_r_neg[:], ins["u_r_neg"][:])
    bias = consts.tile([128, 4], f32, tag="bias")
    nc.sync.dma_start(bias[:], ins["bias"][:])
    b_r, b_z, b_in, b_hn = (bias[:, i : i + 1] for i in range(4))

    w_r, w_z, w_n = (wih[:, g * H : (g + 1) * H] for g in range(3))
    u_r, u_z, u_n = (whh[:, g * H : (g + 1) * H] for g in range(3))

    h_init = consts.tile([128, cols], bf16, tag="hinit")
    nc.sync.dma_start(h_init[:], ins["zeros"][:])

    x_dram = ins["x_t"]
    # h viewed as [128, steps, cols] for strided per-chain pair stores
    h_dram = outs["h_out"].rearrange("p (t c) -> p t c", c=cols)

    # persistent per-chain PSUM banks (4 per chain = 8 total)
    prb = [pRZ.tile([128, cc], f32, tag=f"prb{c}", name=f"prb{c}")
           for c in range(n_chains)]
    pzb = [pRZ.tile([128, cc], f32, tag=f"pzb{c}", name=f"pzb{c}")
           for c in range(n_chains)]
    phn = [pN.tile([128, cc], f32, tag=f"phn{c}", name=f"phn{c}")
           for c in range(n_chains)]
    pB = [pN.tile([128, cc], f32, tag=f"pB{c}", name=f"pB{c}")
          for c in range(n_chains)]

    xtiles = {}
    h_prev = [h_init[:, c * cc : (c + 1) * cc] for c in range(n_chains)]
    # stagger chain 1 by ~half a step period (serial copy chain)
    if n_chains == 2:
        stag = h_prev[1]
        for s in range(STAGGER):
            nxt = consts.tile([128, cc], bf16, tag=f"stag{s}", name=f"stag{s}")
            nc.vector.tensor_copy(nxt[:], stag)
            stag = nxt[:]
        h_prev[1] = stag
    h_pair = [None] * n_chains
    # deferred v-split accumulation operands for the NEXT step's pr
    vu_accum = [None] * n_chains

    def get_block(bp):
        if bp not in xtiles:
            bsteps = min(xblk, steps - bp * xblk)
            xt_blk = xpool.tile([128, bsteps * cols], bf16, tag="xblk",
                                name=f"xblk_{bp}")
            nc.sync.dma_start(
                xt_blk[:], x_dram[:, bp * xblk * cols : (bp * xblk + bsteps) * cols]
            )
            xtiles[bp] = xt_blk
            for stale in [k for k in xtiles if k < bp - 2]:
                del xtiles[stale]
        return xtiles[bp]

    def x_step(tp, c):
        """[128, cc] moving operand of x for step tp, chain c."""
        xt_b = get_block(tp // xblk)
        v = xt_b[:].rearrange("p (s c) -> p s c", c=cols)
        return v[:, tp % xblk, c * cc : (c + 1) * cc]

    for t in range(steps):
        get_block(t // xblk)

        for c in range(n_chains):
            hp = h_prev[c]

            if t % 2 == 0:
                h_pair[c] = hpool.tile([128, 2 * cc], bf16,
                                       tag=f"hpair{c}", name=f"hpair{c}_{t}")

            # x-side projections for this step (reset banks; they only wait
            # on last step's sigmoid reads, well off the critical path)
            xs = x_step(t, c)
            nc.tensor.matmul(prb[c][:], w_r, xs, start=True, stop=False,
                             skip_group_check=True)
            nc.tensor.matmul(pzb[c][:], w_z, xs, start=True, stop=False,
                             skip_group_check=True)

            # r-gate hidden contribution: deferred v/u split from the
            # previous step (or plain U_r @ h on the first step).
            if vu_accum[c] is not None:
                v_prev, u_prev = vu_accum[c]
                nc.tensor.matmul(prb[c][:], u_r, v_prev, start=False,
                                 stop=False, skip_group_check=True)
                nc.tensor.matmul(prb[c][:], u_r_neg[:], u_prev, start=False,
                                 stop=True, skip_group_check=True)
            else:
                nc.tensor.matmul(prb[c][:], u_r, hp, start=False, stop=True,
                                 skip_group_check=True)
            nc.tensor.matmul(phn[c][:], u_n, hp, start=True, stop=True)
            nc.tensor.matmul(pzb[c][:], u_z, hp, start=False, stop=True,
                             skip_group_check=True)

            r_t = spool.tile([128, cc], bf16, tag=f"r{c}")
            nc.scalar.activation(r_t[:], prb[c][:], AF.Sigmoid, bias=b_r)
            z_t = spool.tile([128, cc], bf16, tag=f"z{c}")
            nc.scalar.activation(z_t[:], pzb[c][:], AF.Sigmoid, bias=b_z)

            # B = (phn + b_hn) * r  (single fused DVE op into PSUM)
            nc.vector.scalar_tensor_tensor(pB[c][:], phn[c][:], b_hn, r_t[:],
                                           OP.add, OP.mult)
            # B += W_n @ x_t  (PE accumulate onto DVE-written bank)
            nc.tensor.matmul(pB[c][:], w_n, x_step(t, c), start=False,
                             stop=True, skip_group_check=True)
            # n = tanh(B + b_in)
            n_t = spool.tile([128, cc], bf16, tag=f"n{c}")
            nc.scalar.activation(n_t[:], pB[c][:], AF.Tanh, bias=b_in)

            # zm1 = z - 1 (off critical path, fast tensor_scalar)
            zm1 = spool.tile([128, cc], bf16, tag=f"zm1{c}")
            nc.vector.tensor_scalar_add(zm1[:], z_t[:], -1.0)
            # v = z * h_prev (off critical path). Pool TT contends for SBUF
            # ports with concurrent DVE TT ops (measured 2.5x slowdown on the
            # critical-path u op), but a Pool op overlapping the other
            # chain's STT showed no contention — so only chain 0's v goes to
            # Pool (it phase-aligns with chain 1's STT), chain 1's stays DVE.
            v_t = spool.tile([128, cc], bf16, tag=f"v{c}")
            nc.vector.tensor_mul(v_t[:], z_t[:], hp)

            # u = (z-1) * n ; h' = v - u = z*h + (1-z)*n
            u_t = spool.tile([128, cc], bf16, tag=f"u{c}")
            nc.vector.tensor_mul(u_t[:], zm1[:], n_t[:])
            half = (t % 2) * cc
            h_new = h_pair[c][:, half : half + cc]
            nc.vector.tensor_sub(h_new, v_t[:], u_t[:])
            h_prev[c] = h_new
            vu_accum[c] = (v_t[:], u_t[:])

            if t % 2 == 1:
                hv = h_pair[c][:].rearrange("p (t c) -> p t c", c=cc)
                nc.sync.dma_start(
                    h_dram[:, t - 1 : t + 1, c * cc : (c + 1) * cc], hv
                )

    ctx.close()


def _declare_io(nc, steps, m_chunks):
    import concourse.mybir as mybir

    cols = 32 * m_chunks
    f32 = mybir.dt.float32
    bf16 = mybir.dt.bfloat16
    ins = {
        "x_t": nc.dram_tensor("x_t", [128, steps * cols], bf16, kind="ExternalInput").ap(),
        "wih_t": nc.dram_tensor("wih_t", [128, 3 * H], bf16, kind="ExternalInput").ap(),
        "whh_t": nc.dram_tensor("whh_t", [128, 3 * H], bf16, kind="ExternalInput").ap(),
        "u_r_neg": nc.dram_tensor("u_r_neg", [128, H], bf16, kind="ExternalInput").ap(),
        "bias": nc.dram_tensor("bias", [128, 4], f32, kind="ExternalInput").ap(),
        "zeros": nc.dram_tensor("zeros", [128, cols], bf16, kind="ExternalInput").ap(),
    }
    outs = {
        "h_out": nc.dram_tensor(
            "h_out", [128, steps * cols], bf16, kind="ExternalOutput"
        ).ap(),
    }
    return ins, outs


def build_module(steps=STEPS, m_chunks=M_CHUNKS, n_chains=N_CHAINS):
    import concourse.bacc as bacc
    import concourse.tile as tile

    nc = bacc.Bacc("TRN2", target_bir_lowering=False, debug=False)
    ins, outs = _declare_io(nc, steps, m_chunks)
    with tile.TileContext(nc) as tc:
        build_gru_program(tc, ins, outs, steps, m_chunks, n_chains)
    nc.compile()
    return nc


# ---------------- host-side data prep / assembly ----------------

def chunk_starts(n_segments, c_steps, l_warm):
    """Compute-range start per global segment (clamped at 0)."""
    return [max(0, s * c_steps - l_warm) for s in range(n_segments)]


def prep_core_inputs(x_dir, wih, whh, bih, bhh, core, steps, m_chunks,
                     c_steps, l_warm):
    """Build the input map for one core of one direction.

    x_dir: [B, T, DX] (already time-reversed for the backward direction)
    wih/whh: [3H, {DX,H}], bih/bhh: [3H]
    """
    cols = 32 * m_chunks
    starts = chunk_starts(CORES_PER_DIR * m_chunks, c_steps, l_warm)
    xt = np.empty((128, steps, m_chunks, B), np.float32)
    for j in range(m_chunks):
        g = starts[core * m_chunks + j]
        xt[:, :, j, :] = np.transpose(x_dir[:, g : g + steps, :], (2, 1, 0))
    bias = np.zeros((128, 4), np.float32)
    bias[:, 0] = bih[0:H] + bhh[0:H]          # r
    bias[:, 1] = bih[H : 2 * H] + bhh[H : 2 * H]  # z
    bias[:, 2] = bih[2 * H : 3 * H]           # input-side n bias (tanh bias)
    bias[:, 3] = bhh[2 * H : 3 * H]           # hidden-side n bias (STT scalar)
    bf = ml_dtypes.bfloat16
    return {
        "x_t": np.ascontiguousarray(xt.reshape(128, steps * cols)).astype(bf),
        "wih_t": np.ascontiguousarray(wih.T).astype(bf),     # [DX, 3H]
        "whh_t": np.ascontiguousarray(whh.T).astype(bf),     # [H, 3H]
        "u_r_neg": np.ascontiguousarray(-whh[0:H, :].T).astype(bf),  # [H, H]
        "bias": bias,
        "zeros": np.zeros((128, cols), bf),
    }


def assemble_direction(h_parts, steps, m_chunks, c_steps, l_warm):
    """h_parts: list over CORES_PER_DIR cores of [128, steps*cols] bf16.
    Returns [B, T, H] hidden states for this direction (pre-reversal)."""
    out = np.empty((B, T, H), np.float32)
    for core in range(CORES_PER_DIR):
        hp = np.asarray(h_parts[core]).reshape(H, steps, m_chunks, B)
        for j in range(m_chunks):
            s = core * m_chunks + j
            off = s * c_steps - max(0, s * c_steps - l_warm)  # warmup offset
            seg = hp[:, off : off + c_steps, j, :].astype(np.float32)
            out[:, s * c_steps : (s + 1) * c_steps, :] = np.transpose(seg, (2, 1, 0))
    return out


_COMPILED = {}


def _get_module(steps, m_chunks):
    key = (steps, m_chunks)
    if key not in _COMPILED:
        _COMPILED[key] = build_module(steps, m_chunks)
    return _COMPILED[key]


def make_in_maps(x, W_ih_f, W_hh_f, b_ih_f, b_hh_f, W_ih_b, W_hh_b, b_ih_b,
                 b_hh_b):
    x = np.asarray(x, np.float32)
    x_rev = x[:, ::-1, :]
    in_maps = []
    for core in range(CORES_PER_DIR):
        in_maps.append(prep_core_inputs(
            x, W_ih_f, W_hh_f, b_ih_f, b_hh_f, core,
            STEPS, M_CHUNKS, C_STEPS, L_WARM))
    for core in range(CORES_PER_DIR):
        in_maps.append(prep_core_inputs(
            x_rev, W_ih_b, W_hh_b, b_ih_b, b_hh_b, core,
            STEPS, M_CHUNKS, C_STEPS, L_WARM))
    return in_maps


LAST_RES = None


def kernel(x, W_ih_f, W_hh_f, b_ih_f, b_hh_f, W_ih_b, W_hh_b, b_ih_b, b_hh_b,
           W_fc, b_fc):
    global LAST_RES
    from concourse.bass_utils import run_bass_kernel_spmd

    nc = _get_module(STEPS, M_CHUNKS)
    in_maps = make_in_maps(x, W_ih_f, W_hh_f, b_ih_f, b_hh_f,
                           W_ih_b, W_hh_b, b_ih_b, b_hh_b)
    res = run_bass_kernel_spmd(nc, in_maps, core_ids=list(range(N_CORES)))
    LAST_RES = res

    hf = assemble_direction([res.results[c]["h_out"] for c in range(4)],
                            STEPS, M_CHUNKS, C_STEPS, L_WARM)
    hb_rev = assemble_direction([res.results[c]["h_out"] for c in range(4, 8)],
                                STEPS, M_CHUNKS, C_STEPS, L_WARM)
    hb = hb_rev[:, ::-1, :]
    W_fc = np.asarray(W_fc, np.float32)
    y = hf @ W_fc[:, 0:H].T + hb @ W_fc[:, H : 2 * H].T
    return (y + np.asarray(b_fc, np.float32)).astype(np.float32)


# revision 22
# speedup vs baseline: 1.3139x; 1.0279x over previous
"""Bidirectional GRU classifier kernel for Trainium2 (8 NeuronCores).

Strategy:
  - Direction parallel + time-sharded: cores 0-3 forward GRU, cores 4-7
    backward GRU (forward scan over time-reversed input); single SPMD
    program, per-core differences live in the input data.
  - Each core owns 1024 output steps split into M_CHUNKS=32 chunks processed
    as parallel columns (40 serial steps incl. 8 warmup); 2 anti-phased
    recurrence chains of 512 columns each.
  - bf16 datapath: x, h, r, z, n and all matmul operands bf16 (PE 1 cyc/row,
    DVE 2x_1p on SBUF elementwise ops, half DMA); PSUM accum stays fp32.
  - Per chain-step:
      pr = W_r x_t + U_r v_{t-1} - U_r u_{t-1}   (v-split: no wait on h')
      pz = W_z x_t + U_z h_{t-1}
      r = sigmoid(pr + b_r), z = sigmoid(pz + b_z)       [ACT]
      B = (phn + b_hn) * r                               [DVE STT -> PSUM]
      B += W_n x_t                                       [PE accumulate]
      n = tanh(B + b_in)                                 [ACT]
      zm1 = z - 1                                        [DVE tensor_scalar]
      u = zm1 * n                                        [DVE TT bf16 2x]
      v = z * h_prev                                     [DVE TT]
      h' = v - u = z*h + (1-z)*n                         [DVE TT]
    All elementwise work stays on DVE: Pool (gpsimd) TT ops contend for
    SBUF ports with concurrent DVE ops (measured 2.5x slowdown of the
    critical-path u op), so an "idle" Pool is faster than a busy one.
    STEPS must stay even (h is DMA'd out in 2-step pairs).
  - No FC on device: h streamed out bf16; FC (K=10) + direction sum + b_fc
    run on host.
"""

import sys

sys.path.insert(0, "/opt/trn_rl_repo")

import numpy as np
import ml_dtypes

# Problem constants
B, T, DX, H, K = 32, 4096, 128, 128, 10
N_CORES = 8
CORES_PER_DIR = 4

# Sharding parameters
M_CHUNKS = 32       # chunks per core
N_CHAINS = 2        # independent recurrence chains per core
C_STEPS = 1024 // M_CHUNKS  # output steps per chunk
L_WARM = 8          # warmup steps per chunk
STEPS = C_STEPS + L_WARM    # compute steps per chunk
COLS = 32 * M_CHUNKS        # total columns per step (batch x chunks)
XBLK = 4            # x-stream block: steps per DMA block
STAGGER = 7         # serial copies to anti-phase chain 1


def build_gru_program(tc, ins, outs, steps, m_chunks, n_chains, xblk=XBLK):
    """Emit the Tile program. ins/outs: dict name -> bass.AP (DRAM)."""
    import concourse.mybir as mybir
    from contextlib import ExitStack

    nc = tc.nc
    f32 = mybir.dt.float32
    bf16 = mybir.dt.bfloat16
    cols = 32 * m_chunks            # per step, all chains
    cc = cols // n_chains           # per chain
    AF = mybir.ActivationFunctionType
    OP = mybir.AluOpType

    ctx = ExitStack()
    consts = ctx.enter_context(tc.tile_pool(name="consts", bufs=1))
    xpool = ctx.enter_context(tc.tile_pool(name="xblk", bufs=3))
    hpool = ctx.enter_context(tc.tile_pool(name="hbuf", bufs=3))
    spool = ctx.enter_context(tc.tile_pool(name="work", bufs=2))
    pRZ = ctx.enter_context(tc.tile_pool(name="pRZ", bufs=1, space="PSUM"))
    pN = ctx.enter_context(tc.tile_pool(name="pN", bufs=1, space="PSUM"))

    x_dram = ins["x_t"]

    xtiles = {}

    def get_block(bp):
        if bp not in xtiles:
            bsteps = min(xblk, steps - bp * xblk)
            xt_blk = xpool.tile([128, bsteps * cols], bf16, tag="xblk",
                                name=f"xblk_{bp}")
            nc.sync.dma_start(
                xt_blk[:], x_dram[:, bp * xblk * cols : (bp * xblk + bsteps) * cols]
            )
            xtiles[bp] = xt_blk
            for stale in [k for k in xtiles if k < bp - 2]:
                del xtiles[stale]
        return xtiles[bp]

    # Kick off the step-0 x block first: it gates the first matmul, while
    # the small weight/bias DMAs below easily hide behind it.
    get_block(0)

    # Load weights/constants once (all bf16 except biases)
    wih = consts.tile([128, 3 * H], bf16, tag="wih")
    nc.sync.dma_start(wih[:], ins["wih_t"][:])
    whh = consts.tile([128, 3 * H], bf16, tag="whh")
    nc.sync.dma_start(whh[:], ins["whh_t"][:])
    u_r_neg = consts.tile([128, H], bf16, tag="urneg")
    nc.sync.dma_start(u_r_neg[:], ins["u_r_neg"][:])
    bias = consts.tile([128, 4], f32, tag="bias")
    nc.sync.dma_start(bias[:], ins["bias"][:])
    b_r, b_z, b_in, b_hn = (bias[:, i : i + 1] for i in range(4))

    w_r, w_z, w_n = (wih[:, g * H : (g + 1) * H] for g in range(3))
    u_r, u_z, u_n = (whh[:, g * H : (g + 1) * H] for g in range(3))

    h_init = consts.tile([128, cols], bf16, tag="hinit")
    nc.sync.dma_start(h_init[:], ins["zeros"][:])
    # h viewed as [128, steps, cols] for strided per-chain pair stores
    h_dram = outs["h_out"].rearrange("p (t c) -> p t c", c=cols)

    # persistent per-chain PSUM banks (4 per chain = 8 total)
    prb = [pRZ.tile([128, cc], f32, tag=f"prb{c}", name=f"prb{c}")
           for c in range(n_chains)]
    pzb = [pRZ.tile([128, cc], f32, tag=f"pzb{c}", name=f"pzb{c}")
           for c in range(n_chains)]
    phn = [pN.tile([128, cc], f32, tag=f"phn{c}", name=f"phn{c}")
           for c in range(n_chains)]
    pB = [pN.tile([128, cc], f32, tag=f"pB{c}", name=f"pB{c}")
          for c in range(n_chains)]

    h_prev = [h_init[:, c * cc : (c + 1) * cc] for c in range(n_chains)]
    # stagger chain 1 by ~half a step period (serial copy chain)
    if n_chains == 2:
        stag = h_prev[1]
        for s in range(STAGGER):
            nxt = consts.tile([128, cc], bf16, tag=f"stag{s}", name=f"stag{s}")
            nc.vector.tensor_copy(nxt[:], stag)
            stag = nxt[:]
        h_prev[1] = stag
    h_pair = [None] * n_chains
    # deferred v-split accumulation operands for the NEXT step's pr
    vu_accum = [None] * n_chains

    def x_step(tp, c):
        """[128, cc] moving operand of x for step tp, chain c."""
        xt_b = get_block(tp // xblk)
        v = xt_b[:].rearrange("p (s c) -> p s c", c=cols)
        return v[:, tp % xblk, c * cc : (c + 1) * cc]

    for t in range(steps):
        get_block(t // xblk)

        for c in range(n_chains):
            hp = h_prev[c]

            if t % 2 == 0:
                h_pair[c] = hpool.tile([128, 2 * cc], bf16,
                                       tag=f"hpair{c}", name=f"hpair{c}_{t}")

            # x-side projections for this step (reset banks; they only wait
            # on last step's sigmoid reads, well off the critical path)
            xs = x_step(t, c)
            nc.tensor.matmul(prb[c][:], w_r, xs, start=True, stop=False,
                             skip_group_check=True)
            nc.tensor.matmul(pzb[c][:], w_z, xs, start=True, stop=False,
                             skip_group_check=True)

            # r-gate hidden contribution: deferred v/u split from the
            # previous step (or plain U_r @ h on the first step).
            if vu_accum[c] is not None:
                v_prev, u_prev = vu_accum[c]
                nc.tensor.matmul(prb[c][:], u_r, v_prev, start=False,
                                 stop=False, skip_group_check=True)
                nc.tensor.matmul(prb[c][:], u_r_neg[:], u_prev, start=False,
                                 stop=True, skip_group_check=True)
            else:
                nc.tensor.matmul(prb[c][:], u_r, hp, start=False, stop=True,
                                 skip_group_check=True)
            nc.tensor.matmul(phn[c][:], u_n, hp, start=True, stop=True)
            nc.tensor.matmul(pzb[c][:], u_z, hp, start=False, stop=True,
                             skip_group_check=True)

            r_t = spool.tile([128, cc], bf16, tag=f"r{c}")
            nc.scalar.activation(r_t[:], prb[c][:], AF.Sigmoid, bias=b_r)

            # B = (phn + b_hn) * r  (single fused DVE op into PSUM)
            nc.vector.scalar_tensor_tensor(pB[c][:], phn[c][:], b_hn, r_t[:],
                                           OP.add, OP.mult)
            # B += W_n @ x_t  (PE accumulate onto DVE-written bank)
            nc.tensor.matmul(pB[c][:], w_n, x_step(t, c), start=False,
                             stop=True, skip_group_check=True)
            # n = tanh(B + b_in)
            n_t = spool.tile([128, cc], bf16, tag=f"n{c}")
            nc.scalar.activation(n_t[:], pB[c][:], AF.Tanh, bias=b_in)

            # sigma_z emitted after the n-path: z's consumers have ~1.5us of
            # slack, and late emission keeps the ACT queue from delaying the
            # critical-path tanh behind the other chain's sigma_z.
            z_t = spool.tile([128, cc], bf16, tag=f"z{c}")
            nc.scalar.activation(z_t[:], pzb[c][:], AF.Sigmoid, bias=b_z)

            # zm1 = z - 1 (off critical path, fast tensor_scalar)
            zm1 = spool.tile([128, cc], bf16, tag=f"zm1{c}")
            nc.vector.tensor_scalar_add(zm1[:], z_t[:], -1.0)
            # v = z * h_prev (off critical path; all elementwise stays on
            # DVE — Pool TT contends for SBUF ports with concurrent DVE TT
            # ops, measured 2.5x slowdown of the critical-path u op)
            v_t = spool.tile([128, cc], bf16, tag=f"v{c}")
            nc.vector.tensor_mul(v_t[:], z_t[:], hp)

            # u = (z-1) * n ; h' = v - u = z*h + (1-z)*n
            u_t = spool.tile([128, cc], bf16, tag=f"u{c}")
            nc.vector.tensor_mul(u_t[:], zm1[:], n_t[:])
            half = (t % 2) * cc
            h_new = h_pair[c][:, half : half + cc]
            nc.vector.tensor_sub(h_new, v_t[:], u_t[:])
            h_prev[c] = h_new
            vu_accum[c] = (v_t[:], u_t[:])

            if t % 2 == 1:
                hv = h_pair[c][:].rearrange("p (t c) -> p t c", c=cc)
                nc.sync.dma_start(
                    h_dram[:, t - 1 : t + 1, c * cc : (c + 1) * cc], hv
                )

    ctx.close()


def _declare_io(nc, steps, m_chunks):
    import concourse.mybir as mybir

    cols = 32 * m_chunks
    f32 = mybir.dt.float32
    bf16 = mybir.dt.bfloat16
    ins = {
        "x_t": nc.dram_tensor("x_t", [128, steps * cols], bf16, kind="ExternalInput").ap(),
        "wih_t": nc.dram_tensor("wih_t", [128, 3 * H], bf16, kind="ExternalInput").ap(),
        "whh_t": nc.dram_tensor("whh_t", [128, 3 * H], bf16, kind="ExternalInput").ap(),
        "u_r_neg": nc.dram_tensor("u_r_neg", [128, H], bf16, kind="ExternalInput").ap(),
        "bias": nc.dram_tensor("bias", [128, 4], f32, kind="ExternalInput").ap(),
        "zeros": nc.dram_tensor("zeros", [128, cols], bf16, kind="ExternalInput").ap(),
    }
    outs = {
        "h_out": nc.dram_tensor(
            "h_out", [128, steps * cols], bf16, kind="ExternalOutput"
        ).ap(),
    }
    return ins, outs


def build_module(steps=STEPS, m_chunks=M_CHUNKS, n_chains=N_CHAINS):
    import concourse.bacc as bacc
    import concourse.tile as tile

    nc = bacc.Bacc("TRN2", target_bir_lowering=False, debug=False)
    ins, outs = _declare_io(nc, steps, m_chunks)
    with tile.TileContext(nc) as tc:
        build_gru_program(tc, ins, outs, steps, m_chunks, n_chains)
    nc.compile()
    return nc


# ---------------- host-side data prep / assembly ----------------

def chunk_starts(n_segments, c_steps, l_warm):
    """Compute-range start per global segment (clamped at 0)."""
    return [max(0, s * c_steps - l_warm) for s in range(n_segments)]


def prep_core_inputs(x_dir, wih, whh, bih, bhh, core, steps, m_chunks,
                     c_steps, l_warm):
    """Build the input map for one core of one direction.

    x_dir: [B, T, DX] (already time-reversed for the backward direction)
    wih/whh: [3H, {DX,H}], bih/bhh: [3H]
    """
    cols = 32 * m_chunks
    starts = chunk_starts(CORES_PER_DIR * m_chunks, c_steps, l_warm)
    xt = np.empty((128, steps, m_chunks, B), np.float32)
    for j in range(m_chunks):
        g = starts[core * m_chunks + j]
        xt[:, :, j, :] = np.transpose(x_dir[:, g : g + steps, :], (2, 1, 0))
    bias = np.zeros((128, 4), np.float32)
    bias[:, 0] = bih[0:H] + bhh[0:H]          # r
    bias[:, 1] = bih[H : 2 * H] + bhh[H : 2 * H]  # z
    bias[:, 2] = bih[2 * H : 3 * H]           # input-side n bias (tanh bias)
    bias[:, 3] = bhh[2 * H : 3 * H]           # hidden-side n bias (STT scalar)
    bf = ml_dtypes.bfloat16
    return {
        "x_t": np.ascontiguousarray(xt.reshape(128, steps * cols)).astype(bf),
        "wih_t": np.ascontiguousarray(wih.T).astype(bf),     # [DX, 3H]
        "whh_t": np.ascontiguousarray(whh.T).astype(bf),     # [H, 3H]
        "u_r_neg": np.ascontiguousarray(-whh[0:H, :].T).astype(bf),  # [H, H]
        "bias": bias,
        "zeros": np.zeros((128, cols), bf),
    }


def assemble_direction(h_parts, steps, m_chunks, c_steps, l_warm):
    """h_parts: list over CORES_PER_DIR cores of [128, steps*cols] bf16.
    Returns [B, T, H] hidden states for this direction (pre-reversal)."""
    out = np.empty((B, T, H), np.float32)
    for core in range(CORES_PER_DIR):
        hp = np.asarray(h_parts[core]).reshape(H, steps, m_chunks, B)
        for j in range(m_chunks):
            s = core * m_chunks + j
            off = s * c_steps - max(0, s * c_steps - l_warm)  # warmup offset
            seg = hp[:, off : off + c_steps, j, :].astype(np.float32)
            out[:, s * c_steps : (s + 1) * c_steps, :] = np.transpose(seg, (2, 1, 0))
    return out


_COMPILED = {}


def _get_module(steps, m_chunks):
    key = (steps, m_chunks)
    if key not in _COMPILED:
        _COMPILED[key] = build_module(steps, m_chunks)
    return _COMPILED[key]


def make_in_maps(x, W_ih_f, W_hh_f, b_ih_f, b_hh_f, W_ih_b, W_hh_b, b_ih_b,
                 b_hh_b):
    x = np.asarray(x, np.float32)
    x_rev = x[:, ::-1, :]
    in_maps = []
    for core in range(CORES_PER_DIR):
        in_maps.append(prep_core_inputs(
            x, W_ih_f, W_hh_f, b_ih_f, b_hh_f, core,
            STEPS, M_CHUNKS, C_STEPS, L_WARM))
    for core in range(CORES_PER_DIR):
        in_maps.append(prep_core_inputs(
            x_rev, W_ih_b, W_hh_b, b_ih_b, b_hh_b, core,
            STEPS, M_CHUNKS, C_STEPS, L_WARM))
    return in_maps


LAST_RES = None


def kernel(x, W_ih_f, W_hh_f, b_ih_f, b_hh_f, W_ih_b, W_hh_b, b_ih_b, b_hh_b,
           W_fc, b_fc):
    global LAST_RES
    from concourse.bass_utils import run_bass_kernel_spmd

    nc = _get_module(STEPS, M_CHUNKS)
    in_maps = make_in_maps(x, W_ih_f, W_hh_f, b_ih_f, b_hh_f,
                           W_ih_b, W_hh_b, b_ih_b, b_hh_b)
    res = run_bass_kernel_spmd(nc, in_maps, core_ids=list(range(N_CORES)))
    LAST_RES = res

    hf = assemble_direction([res.results[c]["h_out"] for c in range(4)],
                            STEPS, M_CHUNKS, C_STEPS, L_WARM)
    hb_rev = assemble_direction([res.results[c]["h_out"] for c in range(4, 8)],
                                STEPS, M_CHUNKS, C_STEPS, L_WARM)
    hb = hb_rev[:, ::-1, :]
    W_fc = np.asarray(W_fc, np.float32)
    y = hf @ W_fc[:, 0:H].T + hb @ W_fc[:, H : 2 * H].T
    return (y + np.asarray(b_fc, np.float32)).astype(np.float32)
